# revision 11
# baseline (speedup 1.0000x reference)
"""Trainium2 Bass kernel for nn_Decoder (teacher-forced LSTM decoder).

Sharding: decoder GEMM + GroupNorm group-sharded (core g owns group g = 8
channels); AllToAll redistributes to batch-sharding (4 batches/core) for the
LSTM recurrence and output projection.

Device pipeline per core:
  P0: W_dec slice -> bf16 -> DMA-transpose -> SBUF lhs
  P1: y = x_cond @ W_dec_g^T + b_dec (all 32 batches), GroupNorm stats +
      normalize, AllToAll -> x_loc [64ch, 4batch, 1024pix]
  P1.5: one-hot embedding matmul + input projection -> pre[t, gate, b]
      (bf16, bounced via DRAM in 12 chunks)
  P2: 3072-step LSTM recurrence, two phase-shifted batch-streams (2+2),
      gate-dim on partitions; h trajectory accumulated in SBUF (bf16)
  P3: output projection hs @ W_ro^T + b_ro -> out [4, 256, 3072]
"""
import os
import sys
import numpy as np

sys.path.insert(0, "/opt/trn_rl_repo")

import ml_dtypes

BF16NP = ml_dtypes.bfloat16

B, DIN, DH, NV = 32, 64, 256, 256
SEQ = int(os.environ.get("KN_SEQ", "3072"))
PIX = SEQ // 3      # pixels per channel (1024 full-size)
NC_ = 8            # cores
BL = B // NC_      # batches per core = 4
GROUP = 8 * PIX    # cols per group = 8ch * PIX
NCHUNK = 12
TCH = SEQ // NCHUNK  # steps per chunk
CW = min(512, TCH * BL)   # matmul free-dim slice width in phase 1.5
NSPL = (TCH * BL) // CW   # slices per chunk
TW = CW // BL             # timesteps per slice
EPS = 1e-5
UNROLL = int(os.environ.get("KN_UNROLL", "8"))
STAGGER = os.environ.get("KN_STAGGER", "1") == "1"

_CACHE: dict = {}


def _build_program():
    import concourse.bacc as bacc
    import concourse.bass as bass
    import concourse.mybir as mybir
    import concourse.tile as tile

    F32 = mybir.dt.float32
    BF16 = mybir.dt.bfloat16
    I32 = mybir.dt.int32
    AF = mybir.ActivationFunctionType
    ALU = mybir.AluOpType
    ds = bass.ds

    nc = bacc.Bacc("TRN2", target_bir_lowering=False, debug=False,
                   num_devices=NC_)

    # ---- I/O ----
    xcT = nc.dram_tensor("xcT", [256, 32], BF16, kind="ExternalInput")
    wdec = nc.dram_tensor("wdec", [GROUP, 256], F32, kind="ExternalInput")
    bdec = nc.dram_tensor("bdec", [1, GROUP], F32, kind="ExternalInput")
    gnw = nc.dram_tensor("gnw", [1, 8], F32, kind="ExternalInput")
    gnb = nc.dram_tensor("gnb", [1, 8], F32, kind="ExternalInput")
    idxT = nc.dram_tensor("idxT", [SEQ, BL], BF16, kind="ExternalInput")
    whhT = nc.dram_tensor("whhT", [256, 1024], BF16, kind="ExternalInput")
    wihT = nc.dram_tensor("wihT", [64, 1024], BF16, kind="ExternalInput")
    bgm = nc.dram_tensor("bgm", [128, 8], F32, kind="ExternalInput")
    embw = nc.dram_tensor("embw", [256, 64], BF16, kind="ExternalInput")
    wroT = nc.dram_tensor("wroT", [256, 256], BF16, kind="ExternalInput")
    brom = nc.dram_tensor("brom", [128, 2], F32, kind="ExternalInput")
    outp = nc.dram_tensor("outp", [BL, 256, SEQ], F32, kind="ExternalOutput")

    # ---- internal DRAM ----
    wdec_bf = nc.dram_tensor("wdec_bf", [GROUP, 256], BF16)
    yn_dram = nc.dram_tensor("yn_dram", [B, GROUP], BF16)
    a2a_out = nc.dram_tensor("a2a_out", [B, GROUP], BF16)
    pre_dram = [nc.dram_tensor(f"pre{c}", [128, TCH * 32], BF16)
                for c in range(NCHUNK)]

    with tile.TileContext(nc) as tc:
        with (
            tc.tile_pool(name="persist", bufs=1) as pp,
            tc.tile_pool(name="work", bufs=3) as wk,
            tc.tile_pool(name="small", bufs=2) as sm,
            tc.tile_pool(name="psA", bufs=3, space="PSUM") as psA,
            tc.tile_pool(name="psR", bufs=4, space="PSUM") as psR,
        ):
            xc_sb = pp.tile([128, 64], BF16, tag="xc")
            for k in range(2):
                nc.sync.dma_start(xc_sb[:, k * 32:(k + 1) * 32],
                                  xcT[k * 128:(k + 1) * 128, :])
            ones32 = pp.tile([1, 32], BF16, tag="ones32")
            nc.vector.memset(ones32[:], 1.0)
            ones128 = pp.tile([1, 128], BF16, tag="ones128")
            nc.vector.memset(ones128[:], 1.0)
            x_loc = pp.tile([64, BL * PIX], BF16, tag="xloc")

            # ================= P0/P1: decoder GEMM + GroupNorm ============
            with tc.tile_pool(name="ph1", bufs=1) as p1:
                nc.gpsimd.dma_start(wdec_bf[:], wdec[:])  # f32 -> bf16 cast
                wdecT = p1.tile([128, 2 * GROUP], BF16, tag="wdecT")
                for k in range(2):
                    nc.sync.dma_start_transpose(
                        wdecT[:, k * GROUP:(k + 1) * GROUP],
                        wdec_bf[:, k * 128:(k + 1) * 128])

                bdec_sb = p1.tile([1, GROUP], BF16, tag="bdec")
                nc.gpsimd.dma_start(bdec_sb[:], bdec[:])

                y_sb = p1.tile([32, GROUP], F32, tag="y")
                scr = p1.tile([32, 512], F32, tag="scr")
                s2p = p1.tile([32, GROUP // 512], F32, tag="s2p")
                for n in range(GROUP // 512):
                    ps = psA.tile([128, 512], F32, tag="mm")
                    for k in range(2):
                        nc.tensor.matmul(
                            ps[:32, :],
                            xc_sb[:, k * 32:(k + 1) * 32],
                            wdecT[:, k * GROUP + n * 512:
                                  k * GROUP + (n + 1) * 512],
                            start=(k == 0), stop=False)
                    nc.tensor.matmul(ps[:32, :], ones32[:1, :],
                                     bdec_sb[:1, n * 512:(n + 1) * 512],
                                     start=False, stop=True)
                    nc.vector.tensor_copy(y_sb[:, n * 512:(n + 1) * 512],
                                          ps[:32, :])
                    nc.scalar.activation(scr[:], ps[:32, :], AF.Square,
                                         accum_out=s2p[:, n:n + 1])

                s1 = p1.tile([32, 1], F32, tag="s1")
                s2 = p1.tile([32, 1], F32, tag="s2")
                nc.vector.tensor_reduce(s1[:], y_sb[:], mybir.AxisListType.X,
                                        ALU.add)
                nc.vector.tensor_reduce(s2[:], s2p[:], mybir.AxisListType.X,
                                        ALU.add)
                mu = p1.tile([32, 1], F32, tag="mu")
                var = p1.tile([32, 1], F32, tag="var")
                std = p1.tile([32, 1], F32, tag="std")
                rstd = p1.tile([32, 1], F32, tag="rstd")
                nc.vector.tensor_scalar(mu[:], s1[:], 1.0 / GROUP, None,
                                        ALU.mult)
                nc.vector.tensor_scalar(var[:], s2[:], 1.0 / GROUP, None,
                                        ALU.mult)
                musq = p1.tile([32, 1], F32, tag="musq")
                nc.vector.tensor_tensor(musq[:], mu[:], mu[:], ALU.mult)
                nc.vector.tensor_tensor(var[:], var[:], musq[:], ALU.subtract)
                nc.vector.tensor_scalar(var[:], var[:], EPS, None, ALU.add)
                nc.scalar.activation(std[:], var[:], AF.Sqrt)
                nc.vector.reciprocal(rstd[:], std[:])

                gn_sb = p1.tile([1, 16], F32, tag="gn_sb")
                nc.sync.dma_start(gn_sb[:, 0:8], gnw[:])
                nc.sync.dma_start(gn_sb[:, 8:16], gnb[:])
                gn_bc = p1.tile([32, 16], F32, tag="gn_bc")
                nc.gpsimd.partition_broadcast(gn_bc[:], gn_sb[:])
                alpha = p1.tile([32, 8], F32, tag="alpha")
                beta = p1.tile([32, 8], F32, tag="beta")
                nc.vector.tensor_scalar(alpha[:], gn_bc[:, 0:8], rstd[:],
                                        None, ALU.mult)
                malpha = p1.tile([32, 8], F32, tag="malpha")
                nc.vector.tensor_scalar(malpha[:], alpha[:], mu[:], None,
                                        ALU.mult)
                nc.vector.tensor_tensor(beta[:], gn_bc[:, 8:16], malpha[:],
                                        ALU.subtract)

                yn_sb = p1.tile([32, GROUP], BF16, tag="yn")
                for ch in range(8):
                    nc.vector.tensor_scalar(
                        yn_sb[:, ch * PIX:(ch + 1) * PIX],
                        y_sb[:, ch * PIX:(ch + 1) * PIX],
                        alpha[:, ch:ch + 1], beta[:, ch:ch + 1],
                        ALU.mult, ALU.add)
                nc.sync.dma_start(yn_dram[:], yn_sb[:])
                nc.gpsimd.collective_compute(
                    "AllToAll", ALU.bypass,
                    ins=[yn_dram[:]], outs=[a2a_out[:]],
                    replica_groups=[list(range(NC_))])

                for g in range(8):
                    nc.sync.dma_start(
                        x_loc[g * 8:(g + 1) * 8, :]
                        .rearrange("c (b p) -> c b p", b=BL),
                        a2a_out[g * BL:(g + 1) * BL, :]
                        .rearrange("b (c p) -> c b p", c=8))

            # ================= P1.5: embedding + input projection =========
            iota_i = pp.tile([128, 1], I32, tag="iota_i")
            nc.gpsimd.iota(iota_i[:], pattern=[[0, 1]], base=0,
                           channel_multiplier=1)
            iota0 = pp.tile([128, 1], F32, tag="iota0")
            iota1 = pp.tile([128, 1], F32, tag="iota1")
            nc.vector.tensor_copy(iota0[:], iota_i[:])
            nc.vector.tensor_scalar(iota1[:], iota0[:], 128.0, None, ALU.add)

            emb_sb = pp.tile([128, 128], BF16, tag="emb")
            for k in range(2):
                nc.sync.dma_start(emb_sb[:, k * 64:(k + 1) * 64],
                                  embw[k * 128:(k + 1) * 128, :])
            wih_sb = pp.tile([64, 1024], BF16, tag="wih")
            nc.sync.dma_start(wih_sb[:], wihT[:])
            bg_sb = pp.tile([128, 8], F32, tag="bg")
            nc.sync.dma_start(bg_sb[:], bgm[:])

            with tc.tile_pool(name="pre_w", bufs=2) as pwp:
                for cc in range(NCHUNK):
                    t0 = cc * TCH
                    p0 = t0 % PIX              # pixel offset within channel
                    idx_bf = sm.tile([1, TCH * BL], BF16, tag="idx")
                    nc.sync.dma_start(
                        idx_bf[:],
                        idxT[t0:t0 + TCH, :].rearrange("t b -> (t b)")[None, :])
                    pre_sb = pwp.tile([128, TCH * 32], BF16, tag="pre_sb")
                    inpT = sm.tile([64, TCH * BL], BF16, tag="inpT")
                    for n in range(NSPL):
                        sl = slice(n * CW, (n + 1) * CW)
                        psi = psA.tile([128, 512], F32, tag="mm")
                        nc.tensor.matmul(psi[:, :CW], ones128[:1, :],
                                         idx_bf[:1, sl], start=True, stop=True)
                        oh0 = wk.tile([128, 512], BF16, tag="oh0")
                        oh1 = wk.tile([128, 512], BF16, tag="oh1")
                        nc.vector.tensor_scalar(oh0[:, :CW], psi[:, :CW],
                                                iota0[:], None, ALU.is_equal)
                        nc.vector.tensor_scalar(oh1[:, :CW], psi[:, :CW],
                                                iota1[:], None, ALU.is_equal)
                        pse = psA.tile([128, 512], F32, tag="mm")
                        nc.tensor.matmul(pse[:64, :CW], emb_sb[:, 0:64],
                                         oh0[:, :CW], start=True, stop=False)
                        nc.tensor.matmul(pse[:64, :CW], emb_sb[:, 64:128],
                                         oh1[:, :CW], start=False, stop=True)
                        xs_ap = (x_loc[:, :]
                                 .rearrange("c (b p) -> c p b", b=BL)
                                 [:, p0 + n * TW: p0 + (n + 1) * TW, :])
                        nc.vector.tensor_tensor(
                            inpT[:, sl].rearrange("c (t b) -> c t b", b=BL),
                            pse[:64, :CW].rearrange("c (t b) -> c t b", b=BL),
                            xs_ap, ALU.add)
                    for m in range(8):
                        for n in range(NSPL):
                            sl = slice(n * CW, (n + 1) * CW)
                            psp = psA.tile([128, 512], F32, tag="mm")
                            nc.tensor.matmul(psp[:, :CW],
                                             wih_sb[:, m * 128:(m + 1) * 128],
                                             inpT[:, sl], start=True,
                                             stop=True)
                            out_ap = (pre_sb[:, :]
                                      .rearrange("p (t m b) -> p t m b",
                                                 m=8, b=BL)
                                      [:, n * TW:(n + 1) * TW, m, :])
                            in_ap = psp[:, :CW].rearrange("p (t b) -> p t b",
                                                          b=BL)
                            if m % 2 == 0:
                                nc.scalar.activation(out_ap, in_ap,
                                                     AF.Identity,
                                                     bias=bg_sb[:, m:m + 1])
                            else:
                                nc.vector.tensor_scalar(out_ap, in_ap,
                                                        bg_sb[:, m:m + 1],
                                                        None, ALU.add)
                    nc.sync.dma_start(pre_dram[cc][:], pre_sb[:])

            # ================= P2: LSTM recurrence ========================
            whh_sb = pp.tile([128, 2048], BF16, tag="whh")
            for k in range(2):
                nc.sync.dma_start(whh_sb[:, k * 1024:(k + 1) * 1024],
                                  whhT[k * 128:(k + 1) * 128, :])
            hs_sb = pp.tile([128, (SEQ + 1) * 8], BF16, tag="hs")
            nc.vector.memset(hs_sb[:, 0:8], 0.0)
            hpp = [[pp.tile([128, 4], BF16, tag=f"h{s}{j}", name=f"h{s}{j}")
                    for j in range(2)] for s in range(2)]
            c_t = [pp.tile([128, 4], F32, tag=f"c{s}", name=f"c{s}")
                   for s in range(2)]
            for s in range(2):
                nc.vector.memset(c_t[s][:], 0.0)
                nc.vector.memset(hpp[s][0][:], 0.0)

            def rec_step(cc, iv, u, s, pre_t, hprev, hnext):
                t_base = cc * TCH + u      # + iv at runtime
                psg = psR.tile([128, 16], F32, tag="rec", name="psg")
                for m in range(8):
                    for k in range(2):
                        nc.tensor.matmul(
                            psg[:, m * 2:(m + 1) * 2],
                            whh_sb[:, k * 1024 + m * 128:
                                   k * 1024 + (m + 1) * 128],
                            hprev[:, k * 2:(k + 1) * 2],
                            start=(k == 0), stop=(k == 1))
                gates = sm.tile([128, 16], F32, tag="gates", name="gates")
                pre_ap = (pre_t[:, :]
                          .rearrange("p (t m b) -> p t m b", m=8, b=BL)
                          [:, ds(iv + u, 1), :, s * 2:s * 2 + 2]
                          .rearrange("p o m b -> p (o m) b"))
                nc.vector.tensor_tensor(
                    gates[:].rearrange("p (m b) -> p m b", b=2),
                    psg[:].rearrange("p (m b) -> p m b", b=2),
                    pre_ap, ALU.add)
                sig = sm.tile([128, 12], F32, tag="sig", name="sig")
                tg = sm.tile([128, 4], F32, tag="tg", name="tg")
                nc.scalar.activation(sig[:], gates[:, 0:12], AF.Sigmoid)
                nc.scalar.activation(tg[:], gates[:, 12:16], AF.Tanh)
                t1 = sm.tile([128, 4], F32, tag="t1", name="t1")
                nc.vector.tensor_tensor(t1[:], sig[:, 0:4], tg[:], ALU.mult)
                nc.vector.tensor_tensor(c_t[s][:], c_t[s][:], sig[:, 4:8],
                                        ALU.mult)
                nc.vector.tensor_tensor(c_t[s][:], c_t[s][:], t1[:], ALU.add)
                tc_ = sm.tile([128, 4], F32, tag="tc", name="tc_")
                nc.scalar.activation(tc_[:], c_t[s][:], AF.Tanh)
                nc.vector.tensor_tensor(hnext[:], sig[:, 8:12], tc_[:],
                                        ALU.mult)
                nc.vector.tensor_copy(
                    hs_sb[:, ds(iv * 8 + (t_base + 1) * 8 + s * 4, 4)],
                    hnext[:])

            assert TCH % UNROLL == 0 and UNROLL % 2 == 0
            with tc.tile_pool(name="pre_r", bufs=2) as prp:
                for cc in range(NCHUNK):
                    pre_t = prp.tile([128, TCH * 32], BF16, tag="pre_rd",
                                     name="pre_t")
                    nc.sync.dma_start(pre_t[:], pre_dram[cc][:])
                    with tc.For_i(0, TCH, UNROLL,
                                  staggered_reset=STAGGER) as iv:
                        for u in range(UNROLL):
                            for s in range(2):
                                rec_step(cc, iv, u, s, pre_t,
                                         hpp[s][u % 2], hpp[s][(u + 1) % 2])

            # ================= P3: output projection ======================
            wro_sb = pp.tile([128, 512], BF16, tag="wro")
            for k in range(2):
                nc.sync.dma_start(wro_sb[:, k * 256:(k + 1) * 256],
                                  wroT[k * 128:(k + 1) * 128, :])
            bro_sb = pp.tile([128, 2], F32, tag="bro")
            nc.sync.dma_start(bro_sb[:], brom[:])
            hs_view = hs_sb[:, 8:].rearrange("p (t s h b) -> p s h b t",
                                             s=2, h=2, b=2)
            for v in range(2):
                for tcn in range(SEQ // 128):
                    pso = psA.tile([128, 512], F32, tag="mm", name="pso")
                    for k in range(2):
                        nc.tensor.matmul(
                            pso[:, :],
                            wro_sb[:, k * 256 + v * 128:
                                   k * 256 + (v + 1) * 128],
                            hs_view[:, :, k, :, tcn * 128:(tcn + 1) * 128],
                            start=(k == 0), stop=(k == 1))
                    o_sb = wk.tile([128, 512], F32, tag="o_sb", name="o_sb")
                    nc.scalar.activation(o_sb[:], pso[:], AF.Identity,
                                         bias=bro_sb[:, v:v + 1])
                    nc.sync.dma_start(
                        outp[:, v * 128:(v + 1) * 128,
                             tcn * 128:(tcn + 1) * 128]
                        .rearrange("(s b) p t -> p s b t", s=2),
                        o_sb[:].rearrange("p (s b t) -> p s b t", s=2, b=2))

    nc.compile()
    return nc


def _make_runner():
    if "run" in _CACHE:
        return _CACHE["run"]
    import jax
    import numpy as np
    from jax.sharding import Mesh, PartitionSpec
    try:
        from jax.experimental.shard_map import shard_map
    except ImportError:
        from jax.sharding import shard_map  # newer jax
    from concourse import bass2jax, mybir
    from concourse.bass2jax import _bass_exec_p, partition_id_tensor, \
        install_neuronx_cc_hook

    install_neuronx_cc_hook()
    nc = _build_program()

    partition_name = (nc.partition_id_tensor.name
                      if nc.partition_id_tensor else None)
    in_names, out_names, out_avals, zero_shapes = [], [], [], []
    for alloc in nc.m.functions[0].allocations:
        if not isinstance(alloc, mybir.MemoryLocationSet):
            continue
        name = alloc.memorylocations[0].name
        if alloc.kind == "ExternalInput":
            if name != partition_name:
                in_names.append(name)
        elif alloc.kind == "ExternalOutput":
            shape = tuple(alloc.tensor_shape)
            dtype = mybir.dt.np(alloc.dtype)
            out_names.append(name)
            out_avals.append(jax.core.ShapedArray(shape, dtype))
            zero_shapes.append((shape, dtype))
    n_params = len(in_names)
    n_outs = len(out_names)
    all_in_names = list(in_names) + list(out_names)
    if partition_name is not None:
        all_in_names.append(partition_name)
    donate = tuple(range(n_params, n_params + n_outs))

    def _body(*args):
        operands = list(args)
        if partition_name is not None:
            operands.append(partition_id_tensor())
        outs = _bass_exec_p.bind(
            *operands,
            out_avals=tuple(out_avals),
            in_names=tuple(all_in_names),
            out_names=tuple(out_names),
            lowering_input_output_aliases=(),
            sim_require_finite=True,
            sim_require_nnan=True,
            nc=nc,
        )
        return tuple(outs)

    devices = jax.devices()[:NC_]
    mesh = Mesh(np.asarray(devices), ("core",))
    in_specs = (PartitionSpec("core"),) * (n_params + n_outs)
    out_specs = (PartitionSpec("core"),) * n_outs
    sharded = jax.jit(
        shard_map(_body, mesh=mesh, in_specs=in_specs, out_specs=out_specs,
                  check_rep=False),
        donate_argnums=donate, keep_unused=True)

    def run(in_maps):
        concat_in = [
            np.concatenate([np.asarray(in_maps[c][nm]) for c in range(NC_)],
                           axis=0)
            for nm in in_names
        ]
        concat_zeros = [np.zeros((NC_ * s[0], *s[1:]), d)
                        for (s, d) in zero_shapes]
        out_arrs = sharded(*concat_in, *concat_zeros)
        return [
            {nm: np.asarray(out_arrs[i]).reshape(NC_, *out_avals[i].shape)[c]
             for i, nm in enumerate(out_names)}
            for c in range(NC_)
        ]

    _CACHE["run"] = run
    return run


def _prep_inputs(z_noisy, theta, W_theta, b_theta, W_dec, b_dec, gn_w, gn_b,
                 emb, v_ih, g_ih, v_hh, g_hh, b_ih, b_hh, W_ro, b_ro,
                 x_target):
    f32 = np.float32
    z = np.asarray(z_noisy, f32)
    W_dec = np.ascontiguousarray(np.asarray(W_dec, f32))
    b_dec = np.asarray(b_dec, f32)

    # weight-norm
    v_ih = np.asarray(v_ih, f32); v_hh = np.asarray(v_hh, f32)
    W_ih = np.asarray(g_ih, f32)[:, None] * v_ih / np.sqrt(
        (v_ih * v_ih).sum(1, keepdims=True))
    W_hh = np.asarray(g_hh, f32)[:, None] * v_hh / np.sqrt(
        (v_hh * v_hh).sum(1, keepdims=True))
    b_g = np.asarray(b_ih, f32) + np.asarray(b_hh, f32)

    # gate order permutation i,f,o,g (so sigmoid covers a contiguous block)
    perm = np.concatenate([np.arange(0, 512), np.arange(768, 1024),
                           np.arange(512, 768)])
    whhT = np.ascontiguousarray(W_hh[perm].T).astype(BF16NP)      # [256,1024]
    wihT = np.ascontiguousarray(W_ih[perm].T).astype(BF16NP)      # [64,1024]
    bgm = np.ascontiguousarray(b_g[perm].reshape(8, 128).T)       # [128,8]

    x_cond = z + np.asarray(theta, f32) @ np.asarray(W_theta, f32).T \
        + np.asarray(b_theta, f32)
    xcT = np.ascontiguousarray(x_cond.T).astype(BF16NP)           # [256,32]

    embw = np.asarray(emb, f32).astype(BF16NP)                    # [256,64]
    wroT = np.ascontiguousarray(np.asarray(W_ro, f32).T).astype(BF16NP)
    brom = np.ascontiguousarray(np.asarray(b_ro, f32).reshape(2, 128).T)

    idx = np.asarray(x_target).reshape(B, SEQ)
    idx_shift = np.concatenate(
        [np.full((B, 1), 256, np.int64), idx[:, :-1].astype(np.int64)], 1)

    gn_w = np.asarray(gn_w, f32); gn_b = np.asarray(gn_b, f32)

    in_maps = []
    for c in range(NC_):
        in_maps.append({
            "xcT": xcT,
            "wdec": W_dec[c * GROUP:(c + 1) * GROUP],
            "bdec": b_dec[c * GROUP:(c + 1) * GROUP][None, :],
            "gnw": gn_w[c * 8:(c + 1) * 8][None, :],
            "gnb": gn_b[c * 8:(c + 1) * 8][None, :],
            "idxT": np.ascontiguousarray(
                idx_shift[c * BL:(c + 1) * BL].T).astype(BF16NP),
            "whhT": whhT, "wihT": wihT, "bgm": bgm, "embw": embw,
            "wroT": wroT, "brom": brom,
        })

    return in_maps


def kernel(**inputs):
    in_maps = _prep_inputs(**inputs)
    run = _make_runner()
    results = run(in_maps)
    out = np.stack([results[c]["outp"] for c in range(NC_)], axis=0)
    return np.ascontiguousarray(
        out.reshape(B, NV, 3, 32, 32)).astype(np.float32)
